# revision 14
# baseline (speedup 1.0000x reference)
"""BlockHadamardDPD kernel for 8x Trainium2 NeuronCores (Bass/Tile).

y = ((x reshaped [., 64] @ H64/8) reshaped back) * sign1, permuted by perm, * sign2

Per core (data-parallel over batch): 1 batch of [4096 tokens, 4096 dim].

v2 design (bf16 pipeline; all weights exactly representable in bf16):
  host: x -> bf16 (RN), weights = W1 (Hadamard*signs*route1, entries +-1/8),
  B2/R3 (0/1 Clos routing). Device returns y in bf16; host casts to fp32.
  Per 256-token strip:
    1. one batched transpose-DMA loads x[strip] straight into transposed
       chunk layout xT [128 rows, chunk-major tokens] (no PE transpose pass)
    2. M1: 32 weight-stationary bf16 matmuls (Hadamard + sign fold + P1)
    3. E1 (Act/DVE): PSUM->SBUF cast to bf16 packing (tokpair, chunk, half)
    4. ST1 (DVE stream transpose, 32x32 blocks on fp32-packed token pairs):
       the Clos digit-swap exchange (c,(q<<5)|w) -> (w,(q<<5)|c)
    5. M2: 32 weight-stationary matmuls (P2 routing, 0/1)
    6. E2 + ST2: same eviction/exchange again
    7. M3: 64 data-stationary transpose-mode matmuls (P3, 0/1) producing
       natural-layout PSUM [tok, dim]
    8. gpsimd casting DMA writes PSUM fp32 -> DRAM bf16 directly
  Route factorization B3.T.B2.T.B1 computed host-side by Konig edge coloring
  (same as v1); T is realized by the DVE stream transpose instead of PE.
"""
import sys
sys.path.insert(0, "/opt/trn_rl_repo")
import numpy as np
import ml_dtypes

B, S, D = 8, 4096, 4096
BLOCK = 64
NCORES = 8
C, R = 32, 128          # chunks x rows (dim = C*R)
T = 256                 # tokens per strip
NSTRIP = S // T         # 16
XT_PITCH = 256          # transpose-DMA writes chunk tiles contiguously (HW-fixed)

_cache = {}


def _hadamard(n):
    H = np.array([[1.0]], dtype=np.float64)
    base = np.array([[1.0, 1.0], [1.0, -1.0]], dtype=np.float64)
    while H.shape[0] < n:
        H = np.kron(H, base)
    return H


def _perfect_matching(cnt):
    """cnt: 32x32 nonneg int matrix, row/col sums equal & positive. Return match[c]=d."""
    n = cnt.shape[0]
    match_d = -np.ones(n, np.int64)   # d -> c

    for c in range(n):
        seen = np.zeros(n, bool)

        def try_c(cc):
            for d in range(n):
                if cnt[cc, d] > 0 and not seen[d]:
                    seen[d] = True
                    if match_d[d] < 0 or try_c(match_d[d]):
                        match_d[d] = cc
                        return True
            return False

        assert try_c(c)
    match = np.empty(n, np.int64)
    for d in range(n):
        match[match_d[d]] = d
    return match


def _route(perm):
    """Factor route(i) = position of g-index i in output = o(i), as B3.T.B2.T.B1."""
    N = D
    o = np.empty(N, np.int64)
    o[perm] = np.arange(N)            # element i -> output position o[i]
    src_c = np.arange(N) >> 7
    dst_c = o >> 7
    cnt = np.zeros((C, C), np.int64)
    buckets = [[[] for _ in range(C)] for _ in range(C)]
    for i in range(N):
        cnt[src_c[i], dst_c[i]] += 1
        buckets[src_c[i]][dst_c[i]].append(i)
    colors = np.empty(N, np.int64)
    match_tab = np.empty((R, C), np.int64)   # color x src-chunk -> dst chunk
    elem_tab = np.empty((R, C), np.int64)    # color x src-chunk -> element id
    work = cnt.copy()
    for color in range(R):
        m = _perfect_matching(work)
        match_tab[color] = m
        for c in range(C):
            d = m[c]
            work[c, d] -= 1
            i = buckets[c][d].pop()
            colors[i] = color
            elem_tab[color, c] = i
    # b1: chunk c, local row (i & 127) -> color
    b1 = np.empty((C, R), np.int64)
    for i in range(N):
        b1[i >> 7, i & 127] = colors[i]
    # b2: mid chunk m, row (q<<5)|c -> (q<<5)|match_tab[(q<<5)|m, c]
    b2 = np.empty((C, R), np.int64)
    for m in range(C):
        for q in range(4):
            for c in range(C):
                b2[m, (q << 5) | c] = (q << 5) | match_tab[(q << 5) | m, c]
    # b3: final chunk d, row (q<<5)|m -> o(i) & 127 for i = elem of color (q<<5)|m into d
    b3 = np.empty((C, R), np.int64)
    for color in range(R):
        q, mm = color >> 5, color & 31
        for c in range(C):
            d = match_tab[color, c]
            i = elem_tab[color, c]
            b3[d, (q << 5) | mm] = o[i] & 127
    # verify factorization
    pos = np.empty(N, np.int64)
    for i in range(N):
        c0, r0 = i >> 7, i & 127
        r1 = b1[c0, r0]                       # B1
        q, w = r1 >> 5, r1 & 31
        c1, rr1 = w, (q << 5) | c0            # T
        rr2 = b2[c1, rr1]                     # B2
        q2, w2 = rr2 >> 5, rr2 & 31
        c2, rr3 = w2, (q2 << 5) | c1          # T
        r3 = b3[c2, rr3]                      # B3
        pos[i] = (c2 << 7) | r3
    assert np.array_equal(pos, o), "routing factorization failed"
    return b1, b2, b3


def _build_weights(perm, sign1, sign2):
    b1, b2, b3 = _route(perm.astype(np.int64))
    o = np.empty(D, np.int64)
    o[perm] = np.arange(D)
    w_vec = (sign1.astype(np.float64) * sign2.astype(np.float64)[o])  # w[i]=s1[i]*s2[o(i)]
    H2 = np.zeros((R, R))
    H64 = _hadamard(BLOCK) / np.sqrt(float(BLOCK))
    H2[:64, :64] = H64
    H2[64:, 64:] = H64
    # W1 lhsT[k, r'] = w[c*128 + il] * H2[il, k],  il = b1_c^-1(r')
    W1 = np.zeros((C, R, R), np.float32)
    for c in range(C):
        inv1 = np.empty(R, np.int64)
        inv1[b1[c]] = np.arange(R)
        for rp in range(R):
            il = inv1[rp]
            W1[c, :, rp] = (w_vec[c * R + il] * H2[il, :]).astype(np.float32)
    # B2 lhsT[k, r''] = 1 if r'' == b2_m(k)
    B2 = np.zeros((C, R, R), np.float32)
    for m in range(C):
        B2[m, np.arange(R), b2[m]] = 1.0
    # R3 rhs[k, n] = 1 if n == b3_d(k)
    R3 = np.zeros((C, R, R), np.float32)
    for d in range(C):
        R3[d, np.arange(R), b3[d]] = 1.0
    return W1, B2, R3


def _build_nc():
    import concourse.bacc as bacc
    import concourse.mybir as mybir
    import concourse.tile_utils as tile_utils
    tile_utils.max_sbuf_usage = 206 * 1024
    from concourse.tile import TileContext

    f32 = mybir.dt.float32
    bf16 = mybir.dt.bfloat16
    nc = bacc.Bacc("TRN2", target_bir_lowering=False, debug=False, num_devices=NCORES)
    x = nc.dram_tensor("x", [S, D], bf16, kind="ExternalInput")
    w1 = nc.dram_tensor("w1", [R, C * R], bf16, kind="ExternalInput")
    b2t = nc.dram_tensor("b2t", [R, C * R], bf16, kind="ExternalInput")
    r3t = nc.dram_tensor("r3t", [R, C * R], bf16, kind="ExternalInput")
    y = nc.dram_tensor("y", [S, D], bf16, kind="ExternalOutput")

    with TileContext(nc) as tc:
        with tc.tile_pool(name="wpool", bufs=1) as wp, \
             tc.tile_pool(name="xin", bufs=2) as xin, \
             tc.tile_pool(name="mid", bufs=2) as mid, \
             tc.tile_pool(name="yout", bufs=2) as yout, \
             tc.tile_pool(name="ps1", bufs=2, space="PSUM") as ps1, \
             tc.tile_pool(name="ps2", bufs=2, space="PSUM") as ps2, \
             tc.tile_pool(name="ps3", bufs=2, space="PSUM") as ps3:
            w1s = wp.tile([R, C * R], bf16, tag="w1s", name="w1s")
            b2s = wp.tile([R, C * R], bf16, tag="b2s", name="b2s")
            r3s = wp.tile([R, C * R], bf16, tag="r3s", name="r3s")
            nc.sync.dma_start(out=w1s[:, :], in_=w1.ap()[:, :])
            nc.sync.dma_start(out=b2s[:, :], in_=b2t.ap()[:, :])
            nc.sync.dma_start(out=r3s[:, :], in_=r3t.ap()[:, :])

            for s in range(NSTRIP):
                t0 = s * T
                # 1. transpose-DMA in: x[t0:t0+T, :] -> xT[r, c*XT_PITCH + t]
                xT = xin.tile([R, C * XT_PITCH], bf16, tag="xT", name=f"xT{s}")
                xT3d = xT[:, :].rearrange("p (c t) -> p c t", c=C)[:, :, :T]
                nc.sync.dma_start_transpose(xT3d, x.ap()[t0:t0 + T, :])

                # 2+3. M1 + E1 (psum -> A1 packed; pair = tokens (t2, t2+128),
                # i.e. bf16 half h selects the 128-token block tb)
                A1 = mid.tile([R, C * T], bf16, tag="A1", name=f"A1{s}")
                A1v = A1[:, :].rearrange("p (t2 c h) -> p c h t2", c=C, h=2)
                for cp in range(C // 4):
                    pm = ps1.tile([R, 4 * T], f32, tag="pm1", name=f"pm1_{s}_{cp}")
                    for j in range(4):
                        c = cp * 4 + j
                        nc.tensor.matmul(pm[:, j * T:(j + 1) * T],
                                         w1s[:, c * R:(c + 1) * R],
                                         xT[:, c * XT_PITCH:c * XT_PITCH + T])
                    pmv = pm[:, :].rearrange("p (cc tb t2) -> p cc tb t2", cc=4, tb=2)
                    nc.scalar.copy(out=A1v[:, cp * 4:cp * 4 + 4], in_=pmv)

                # 4. ST1: exchange (c,(q<<5)|w) -> (w,(q<<5)|c) on fp32 pairs
                Z2 = mid.tile([R, C * T], bf16, tag="Z2", name=f"Z2{s}")
                nc.vector.transpose(Z2[:, :].bitcast(f32), A1[:, :].bitcast(f32))

                # 5+6. M2 (normal mode, 0/1 weights exact) + E2 -> A2 (t2, w, h)
                A2 = mid.tile([R, C * T], bf16, tag="A2", name=f"A2{s}")
                A2v = A2[:, :].rearrange("p (t2 w h) -> p w h t2", w=C, h=2)
                Z2v = Z2[:, :].rearrange("p (t2 w h) -> p w h t2", w=C, h=2)
                for wp_ in range(C // 2):
                    pm = ps2.tile([R, 2 * T], f32, tag="pm2", name=f"pm2_{s}_{wp_}")
                    for j in range(2):
                        w = wp_ * 2 + j
                        nc.tensor.matmul(pm[:, j * T:(j + 1) * T],
                                         b2s[:, w * R:(w + 1) * R],
                                         Z2v[:, w])
                    pmv = pm[:, :].rearrange("p (cc tb t2) -> p cc tb t2", cc=2, tb=2)
                    if wp_ < 13:
                        nc.scalar.copy(out=A2v[:, wp_ * 2:wp_ * 2 + 2], in_=pmv)
                    else:
                        nc.vector.tensor_copy(A2v[:, wp_ * 2:wp_ * 2 + 2], pmv)

                # 7. ST2: exchange (w,(q2<<5)|w2) -> (w2,(q2<<5)|w)
                Z3 = mid.tile([R, C * T], bf16, tag="Z3", name=f"Z3{s}")
                nc.vector.transpose(Z3[:, :].bitcast(f32), A2[:, :].bitcast(f32))

                # 8+9. M3 (data-stationary, transpose mode, bf16 psum natural
                # [tok, dim]) + E3 eviction into natural tile + DMA out
                Z3v = Z3[:, :].rearrange("p (t2 d h) -> p d h t2", d=C, h=2)
                for tb in range(T // R):
                    ynat = yout.tile([R, D], bf16, tag="ynat",
                                     name=f"ynat{s}_{tb}")
                    for d8 in range(C // 8):
                        pm = ps3.tile([R, 8 * R], bf16, tag="pm3",
                                      name=f"pm3_{s}_{tb}_{d8}")
                        for j in range(8):
                            d = d8 * 8 + j
                            lhs = Z3v[:, d, tb]
                            nc.tensor.matmul(pm[:, j * R:(j + 1) * R],
                                             lhs,
                                             r3s[:, d * R:(d + 1) * R],
                                             is_transpose=True)
                        dst = ynat[:, d8 * 8 * R:(d8 + 1) * 8 * R]
                        nc.vector.tensor_copy(dst, pm[:, :])
                    nc.sync.dma_start(
                        out=y.ap()[t0 + tb * R:t0 + (tb + 1) * R, :],
                        in_=ynat[:, :])
    nc.compile()
    return nc


def _prep_inputs(perm, sign1, sign2):
    W1, B2, R3 = _build_weights(np.asarray(perm), np.asarray(sign1), np.asarray(sign2))
    w1p = np.concatenate([W1[c] for c in range(C)], axis=1).astype(ml_dtypes.bfloat16)
    b2p = np.concatenate([B2[c] for c in range(C)], axis=1).astype(ml_dtypes.bfloat16)
    r3p = np.concatenate([R3[c] for c in range(C)], axis=1).astype(ml_dtypes.bfloat16)
    return w1p, b2p, r3p


def kernel(x, sign1, sign2, perm):
    key = (perm.tobytes(), sign1.tobytes(), sign2.tobytes())
    if key not in _cache:
        w1p, b2p, r3p = _prep_inputs(np.asarray(perm), np.asarray(sign1),
                                     np.asarray(sign2))
        nc = _build_nc()
        _cache[key] = (nc, w1p, b2p, r3p)
    nc, w1p, b2p, r3p = _cache[key]

    from concourse.bass_utils import run_bass_kernel_spmd
    xb = np.asarray(x).astype(ml_dtypes.bfloat16)
    in_maps = [{"x": xb[b], "w1": w1p, "b2t": b2p, "r3t": r3p} for b in range(B)]
    res = run_bass_kernel_spmd(nc, in_maps, list(range(NCORES)))
    out = np.stack([res.results[b]["y"] for b in range(B)], axis=0)
    return out.astype(np.float32)


# revision 15
# speedup vs baseline: 2.3047x; 2.3047x over previous
"""BlockHadamardDPD kernel for 8x Trainium2 NeuronCores (Bass/Tile).

y = ((x reshaped [., 64] @ H64/8) reshaped back) * sign1, permuted by perm, * sign2

Per core (data-parallel over batch): 1 batch of [4096 tokens, 4096 dim].

v2 design (bf16 pipeline; all weights exactly representable in bf16):
  host: x -> bf16 (RN), weights = W1 (Hadamard*signs*route1, entries +-1/8),
  B2/R3 (0/1 Clos routing). Device returns y in bf16; host casts to fp32.
  Per 256-token strip:
    1. one batched transpose-DMA loads x[strip] straight into transposed
       chunk layout xT [128 rows, chunk-major tokens] (no PE transpose pass)
    2. M1: 32 weight-stationary bf16 matmuls (Hadamard + sign fold + P1)
    3. E1 (Act/DVE): PSUM->SBUF cast to bf16 packing (tokpair, chunk, half)
    4. ST1 (DVE stream transpose, 32x32 blocks on fp32-packed token pairs):
       the Clos digit-swap exchange (c,(q<<5)|w) -> (w,(q<<5)|c)
    5. M2: 32 weight-stationary matmuls (P2 routing, 0/1)
    6. E2 + ST2: same eviction/exchange again
    7. M3: 64 data-stationary transpose-mode matmuls (P3, 0/1) producing
       natural-layout PSUM [tok, dim]
    8. gpsimd casting DMA writes PSUM fp32 -> DRAM bf16 directly
  Route factorization B3.T.B2.T.B1 computed host-side by Konig edge coloring
  (same as v1); T is realized by the DVE stream transpose instead of PE.
"""
import sys
sys.path.insert(0, "/opt/trn_rl_repo")
import numpy as np
import ml_dtypes

B, S, D = 8, 4096, 4096
BLOCK = 64
NCORES = 8
C, R = 32, 128          # chunks x rows (dim = C*R)
T = 256                 # tokens per strip
NSTRIP = S // T         # 16
XT_PITCH = 256          # transpose-DMA writes chunk tiles contiguously (HW-fixed)

_cache = {}


def _hadamard(n):
    H = np.array([[1.0]], dtype=np.float64)
    base = np.array([[1.0, 1.0], [1.0, -1.0]], dtype=np.float64)
    while H.shape[0] < n:
        H = np.kron(H, base)
    return H


def _perfect_matching(cnt):
    """cnt: 32x32 nonneg int matrix, row/col sums equal & positive. Return match[c]=d."""
    n = cnt.shape[0]
    match_d = -np.ones(n, np.int64)   # d -> c

    for c in range(n):
        seen = np.zeros(n, bool)

        def try_c(cc):
            for d in range(n):
                if cnt[cc, d] > 0 and not seen[d]:
                    seen[d] = True
                    if match_d[d] < 0 or try_c(match_d[d]):
                        match_d[d] = cc
                        return True
            return False

        assert try_c(c)
    match = np.empty(n, np.int64)
    for d in range(n):
        match[match_d[d]] = d
    return match


def _route(perm):
    """Factor route(i) = position of g-index i in output = o(i), as B3.T.B2.T.B1."""
    N = D
    o = np.empty(N, np.int64)
    o[perm] = np.arange(N)            # element i -> output position o[i]
    src_c = np.arange(N) >> 7
    dst_c = o >> 7
    cnt = np.zeros((C, C), np.int64)
    buckets = [[[] for _ in range(C)] for _ in range(C)]
    for i in range(N):
        cnt[src_c[i], dst_c[i]] += 1
        buckets[src_c[i]][dst_c[i]].append(i)
    colors = np.empty(N, np.int64)
    match_tab = np.empty((R, C), np.int64)   # color x src-chunk -> dst chunk
    elem_tab = np.empty((R, C), np.int64)    # color x src-chunk -> element id
    work = cnt.copy()
    for color in range(R):
        m = _perfect_matching(work)
        match_tab[color] = m
        for c in range(C):
            d = m[c]
            work[c, d] -= 1
            i = buckets[c][d].pop()
            colors[i] = color
            elem_tab[color, c] = i
    # b1: chunk c, local row (i & 127) -> color
    b1 = np.empty((C, R), np.int64)
    for i in range(N):
        b1[i >> 7, i & 127] = colors[i]
    # b2: mid chunk m, row (q<<5)|c -> (q<<5)|match_tab[(q<<5)|m, c]
    b2 = np.empty((C, R), np.int64)
    for m in range(C):
        for q in range(4):
            for c in range(C):
                b2[m, (q << 5) | c] = (q << 5) | match_tab[(q << 5) | m, c]
    # b3: final chunk d, row (q<<5)|m -> o(i) & 127 for i = elem of color (q<<5)|m into d
    b3 = np.empty((C, R), np.int64)
    for color in range(R):
        q, mm = color >> 5, color & 31
        for c in range(C):
            d = match_tab[color, c]
            i = elem_tab[color, c]
            b3[d, (q << 5) | mm] = o[i] & 127
    # verify factorization
    pos = np.empty(N, np.int64)
    for i in range(N):
        c0, r0 = i >> 7, i & 127
        r1 = b1[c0, r0]                       # B1
        q, w = r1 >> 5, r1 & 31
        c1, rr1 = w, (q << 5) | c0            # T
        rr2 = b2[c1, rr1]                     # B2
        q2, w2 = rr2 >> 5, rr2 & 31
        c2, rr3 = w2, (q2 << 5) | c1          # T
        r3 = b3[c2, rr3]                      # B3
        pos[i] = (c2 << 7) | r3
    assert np.array_equal(pos, o), "routing factorization failed"
    return b1, b2, b3


def _build_weights(perm, sign1, sign2):
    b1, b2, b3 = _route(perm.astype(np.int64))
    o = np.empty(D, np.int64)
    o[perm] = np.arange(D)
    w_vec = (sign1.astype(np.float64) * sign2.astype(np.float64)[o])  # w[i]=s1[i]*s2[o(i)]
    H2 = np.zeros((R, R))
    H64 = _hadamard(BLOCK) / np.sqrt(float(BLOCK))
    H2[:64, :64] = H64
    H2[64:, 64:] = H64
    # W1 lhsT[k, r'] = w[c*128 + il] * H2[il, k],  il = b1_c^-1(r')
    W1 = np.zeros((C, R, R), np.float32)
    for c in range(C):
        inv1 = np.empty(R, np.int64)
        inv1[b1[c]] = np.arange(R)
        for rp in range(R):
            il = inv1[rp]
            W1[c, :, rp] = (w_vec[c * R + il] * H2[il, :]).astype(np.float32)
    # B2 lhsT[k, r''] = 1 if r'' == b2_m(k)
    B2 = np.zeros((C, R, R), np.float32)
    for m in range(C):
        B2[m, np.arange(R), b2[m]] = 1.0
    # R3 rhs[k, n] = 1 if n == b3_d(k)
    R3 = np.zeros((C, R, R), np.float32)
    for d in range(C):
        R3[d, np.arange(R), b3[d]] = 1.0
    return W1, B2, R3


def _build_nc():
    import concourse.bacc as bacc
    import concourse.mybir as mybir
    import concourse.tile_utils as tile_utils
    tile_utils.max_sbuf_usage = 206 * 1024
    from concourse.tile import TileContext

    f32 = mybir.dt.float32
    bf16 = mybir.dt.bfloat16
    nc = bacc.Bacc("TRN2", target_bir_lowering=False, debug=False, num_devices=NCORES)
    x = nc.dram_tensor("x", [S, D], bf16, kind="ExternalInput")
    w1 = nc.dram_tensor("w1", [R, C * R], bf16, kind="ExternalInput")
    b2t = nc.dram_tensor("b2t", [R, C * R], bf16, kind="ExternalInput")
    r3t = nc.dram_tensor("r3t", [R, C * R], bf16, kind="ExternalInput")
    y = nc.dram_tensor("y", [S, D], bf16, kind="ExternalOutput")

    with TileContext(nc) as tc:
        with tc.tile_pool(name="wpool", bufs=1) as wp, \
             tc.tile_pool(name="xin", bufs=2) as xin, \
             tc.tile_pool(name="mid", bufs=2) as mid, \
             tc.tile_pool(name="yout", bufs=2) as yout, \
             tc.tile_pool(name="ps1", bufs=2, space="PSUM") as ps1, \
             tc.tile_pool(name="ps2", bufs=2, space="PSUM") as ps2, \
             tc.tile_pool(name="ps3", bufs=2, space="PSUM") as ps3:
            w1s = wp.tile([R, C * R], bf16, tag="w1s", name="w1s")
            b2s = wp.tile([R, C * R], bf16, tag="b2s", name="b2s")
            r3s = wp.tile([R, C * R], bf16, tag="r3s", name="r3s")
            nc.sync.dma_start(out=w1s[:, :], in_=w1.ap()[:, :])
            nc.sync.dma_start(out=b2s[:, :], in_=b2t.ap()[:, :])
            nc.sync.dma_start(out=r3s[:, :], in_=r3t.ap()[:, :])

            for s in range(NSTRIP):
                t0 = s * T
                # 1. transpose-DMA in: x[t0:t0+T, :] -> xT[r, c*T + t]
                # (the lowering writes the 32 transposed chunk tiles
                # contiguously; chunk stride is exactly T)
                xT = xin.tile([R, C * T], bf16, tag="xT", name=f"xT{s}")
                xT3d = xT[:, :].rearrange("p (c t) -> p c t", c=C)
                nc.sync.dma_start_transpose(xT3d, x.ap()[t0:t0 + T, :])
                # rhs column order (t2, h): psum col n <-> token (n&1)*128+(n>>1)
                # so evictions are plain contiguous copies and the bf16 pair
                # packing (t, t+128) falls out of the storage order.
                xTv = xT[:, :].rearrange("p (c h t2) -> p c t2 h", c=C, h=2)

                # 2+3. M1 + E1 (contiguous cast copy -> A1 chunk-major)
                A1 = mid.tile([R, C * T], bf16, tag="A1", name=f"A1{s}")
                for cp in range(C // 4):
                    pm = ps1.tile([R, 4 * T], f32, tag="pm1", name=f"pm1_{s}_{cp}")
                    for j in range(4):
                        c = cp * 4 + j
                        nc.tensor.matmul(pm[:, j * T:(j + 1) * T],
                                         w1s[:, c * R:(c + 1) * R],
                                         xTv[:, c])
                    nc.scalar.copy(out=A1[:, cp * 4 * T:(cp + 1) * 4 * T],
                                   in_=pm[:, :])

                # 4. ST1: exchange (c,(q<<5)|w) -> (w,(q<<5)|c) on fp32 pairs;
                # (t2 outer, chunk inner) enumeration over chunk-major tiles
                Z2 = mid.tile([R, C * T], bf16, tag="Z2", name=f"Z2{s}")
                nc.vector.transpose(
                    Z2[:, :].bitcast(f32).rearrange("p (w t2) -> p t2 w", w=C),
                    A1[:, :].bitcast(f32).rearrange("p (c t2) -> p t2 c", c=C))

                # 5+6. M2 (normal mode, 0/1 weights exact) + E2 contiguous
                A2 = mid.tile([R, C * T], bf16, tag="A2", name=f"A2{s}")
                for wp_ in range(C // 2):
                    pm = ps2.tile([R, 2 * T], f32, tag="pm2", name=f"pm2_{s}_{wp_}")
                    for j in range(2):
                        w = wp_ * 2 + j
                        nc.tensor.matmul(pm[:, j * T:(j + 1) * T],
                                         b2s[:, w * R:(w + 1) * R],
                                         Z2[:, w * T:(w + 1) * T])
                    dst = A2[:, wp_ * 2 * T:(wp_ + 1) * 2 * T]
                    if wp_ % 4 < 3:
                        nc.scalar.copy(out=dst, in_=pm[:, :])
                    else:
                        nc.vector.tensor_copy(dst, pm[:, :])

                # 7. ST2: exchange (w,(q2<<5)|w2) -> (w2,(q2<<5)|w)
                Z3 = mid.tile([R, C * T], bf16, tag="Z3", name=f"Z3{s}")
                nc.vector.transpose(
                    Z3[:, :].bitcast(f32).rearrange("p (d t2) -> p t2 d", d=C),
                    A2[:, :].bitcast(f32).rearrange("p (w t2) -> p t2 w", w=C))

                # 8+9. M3 (data-stationary, transpose mode, bf16 psum natural
                # [tok, dim]) + E3 eviction into natural tile + DMA out
                Z3v = Z3[:, :].rearrange("p (d t2 h) -> p d h t2", d=C, h=2)
                for tb in range(T // R):
                    ynat = yout.tile([R, D], bf16, tag="ynat",
                                     name=f"ynat{s}_{tb}")
                    for d8 in range(C // 8):
                        pm = ps3.tile([R, 8 * R], bf16, tag="pm3",
                                      name=f"pm3_{s}_{tb}_{d8}")
                        for j in range(8):
                            d = d8 * 8 + j
                            lhs = Z3v[:, d, tb]
                            nc.tensor.matmul(pm[:, j * R:(j + 1) * R],
                                             lhs,
                                             r3s[:, d * R:(d + 1) * R],
                                             is_transpose=True)
                        dst = ynat[:, d8 * 8 * R:(d8 + 1) * 8 * R]
                        nc.vector.tensor_copy(dst, pm[:, :])
                    nc.sync.dma_start(
                        out=y.ap()[t0 + tb * R:t0 + (tb + 1) * R, :],
                        in_=ynat[:, :])
    nc.compile()
    return nc


def _prep_inputs(perm, sign1, sign2):
    W1, B2, R3 = _build_weights(np.asarray(perm), np.asarray(sign1), np.asarray(sign2))
    w1p = np.concatenate([W1[c] for c in range(C)], axis=1).astype(ml_dtypes.bfloat16)
    b2p = np.concatenate([B2[c] for c in range(C)], axis=1).astype(ml_dtypes.bfloat16)
    r3p = np.concatenate([R3[c] for c in range(C)], axis=1).astype(ml_dtypes.bfloat16)
    return w1p, b2p, r3p


def kernel(x, sign1, sign2, perm):
    key = (perm.tobytes(), sign1.tobytes(), sign2.tobytes())
    if key not in _cache:
        w1p, b2p, r3p = _prep_inputs(np.asarray(perm), np.asarray(sign1),
                                     np.asarray(sign2))
        nc = _build_nc()
        _cache[key] = (nc, w1p, b2p, r3p)
    nc, w1p, b2p, r3p = _cache[key]

    from concourse.bass_utils import run_bass_kernel_spmd
    xb = np.asarray(x).astype(ml_dtypes.bfloat16)
    in_maps = [{"x": xb[b], "w1": w1p, "b2t": b2p, "r3t": r3p} for b in range(B)]
    res = run_bass_kernel_spmd(nc, in_maps, list(range(NCORES)))
    out = np.stack([res.results[b]["y"] for b in range(B)], axis=0)
    return out.astype(np.float32)
